# revision 14
# baseline (speedup 1.0000x reference)
"""Trainium2 Bass kernel for nn_BBoxGenerator (segment_reduce).

mask_fg (256, 1, 512, 512) f32 -> boxes (256, 4) f32 [x0, y0, x1, y1].

Pure data parallel: each of the 8 cores handles 32 images independently.

v13 (on top of v12's HWDGE f32 stream):
  Trace analysis of v12 (103-110us): the DMA stream itself sits at the
  per-NC HBM roofline (~94us incl. contention with sibling NCs), but the
  measured window (first framework MEMSET -> last epilogue instruction)
  charges ~2.4us of pre-stream dead time and ~15us after the last input
  byte: compute tail ~5.6us + out-DMA ~1.4us + a fixed ~9us framework
  barrier/sem-reset ceremony. v13 attacks the two movable pieces:

  - Earlier stream start: the first three DMA triggers (image 0 pieces +
    pairs (1,2),(3,4)) issue on the ACT HWDGE ring (nc.scalar) - the Sync
    ring's framework preamble (2nd barrier + 703ns DRAIN) delays its
    first trigger to ~7.2us while ACT can trigger at ~6.2us.  ~1us.
  - Shorter compute tail after the last byte:
    * iota tables pre-scaled by 1/512 (exact in f32; <=1px err in bf16),
      so reduce outputs are already normalized; the +1/-1/512 offsets
      fold into the box algebra. hi-side tables are NEGATED so every
      extreme is a MIN; B = -braw_hi.
    * row side: full transpose T1 + reduces for images 0..29 run
      mid-stream (after group(24,6) at image 29); the tail only
      transposes/reduces images 30,31 (T2, 2 small PSUM reduces).
    * image 31's last row block is thresholded in W-halves on DVE and
      ACT concurrently (two accum cells summed on GpSimd), feeding two
      half-width matmuls; each carries stop for its PSUM columns.
    * col side: (psum > thr) in W-halves - DVE is_gt {0,1} and ACT
      Sign(x-thr) {-1,+1}; with wm_lo<0 and wm_hi_neg<0 the min still
      selects correctly and empty gives 0 (ACT-none contributes positive
      values, ignored by min). One (B,2,512) mult (colany broadcast
      against the combined [wm_lo|wm_hi_neg] table) + ONE fused min
      reduce into braw cols {0,2} (stride-2 out AP).
    * box finishing: c2 = A + (1-1/512) - negB in one stt; lo path on
      DVE, hi path + x0/x1/emp4 on GpSimd; empty-default applied with
      one copy_predicated using a broadcast (B,4) mask.
"""

import numpy as np

from concourse import bacc, mybir
from concourse.tile import TileContext
from concourse.bass_utils import run_bass_kernel_spmd

F32 = mybir.dt.float32
BF16 = mybir.dt.bfloat16
I32 = mybir.dt.int32
OP = mybir.AluOpType
AX = mybir.AxisListType
AF = mybir.ActivationFunctionType

N_CORES = 8
B = 256
BP = B // N_CORES  # 32 images per core
H = W = 512
IMG_FREE = 4 * W  # 2048 free elems per image (4 rows per partition)
GROUP = 8
OHW = 64  # one-hot block width per image (rows 32..63 unused)

MIN_BOX = 0.05
ANY_THR = 0.002  # any_t/PSUM sums exceed this iff any foreground
INV = 1.0 / 512
EMP_THR = 0.001  # nonempty y gives B_y >= 1/512, i.e. negB_y <= -1/512


def build_nc():
    nc = bacc.Bacc("TRN2", target_bir_lowering=False, debug=False, num_devices=N_CORES)
    x = nc.declare_dram_parameter("mask_fg", [BP, 1, H, W], F32, isOutput=False)
    out = nc.declare_dram_parameter("out", [BP, 4], F32, isOutput=True)

    # (128, BP, 4, 512): partition p holds rows 4p..4p+3 of each image
    xv = x.ap().rearrange("b one (p a) w -> p (b one) a w", p=128)

    with TileContext(nc) as tc:
        with (
            tc.tile_pool(name="imgs", bufs=16) as imgs,
            tc.tile_pool(name="masks", bufs=6) as masks,
            tc.tile_pool(name="small", bufs=1) as small,
            tc.tile_pool(name="pcol", bufs=1, space="PSUM") as pcol_pool,
            tc.tile_pool(name="ptr", bufs=2, space="PSUM") as ptr_pool,
        ):
            psum_col = pcol_pool.tile([OHW, W], F32)
            oh = small.tile([128, BP * OHW], BF16)
            ones_oh = small.tile([128, BP * OHW], BF16)
            any_t = small.tile([128, 4 * BP], F32)
            any_ab = small.tile([128, 2], F32)
            rvals = small.tile([128, 2 * BP], F32)

            neg_half = small.tile([128, 1], F32)
            neg_thr = small.tile([128, 1], F32)
            b_emp = small.tile([128, 1], F32)  # emp4 Relu bias (needs AP)
            hm_lo_i = small.tile([128, GROUP * 4], I32)
            hm_lo_f = small.tile([128, GROUP * 4], F32)
            hm_lo = small.tile([128, GROUP * 4], F32)
            hm_hi_i = small.tile([128, GROUP * 4], I32)
            hm_hi_f = small.tile([128, GROUP * 4], F32)
            hm_hi = small.tile([128, GROUP * 4], F32)
            # combined col-index table: [:, 0:W] = (j-512)/512 (lo),
            # [:, W:2W] = -(j+1)/512 (negated hi); both < 0
            wm_i = small.tile([BP, 2 * W], I32)
            wm_f = small.tile([BP, 2 * W], F32)
            wm2 = small.tile([BP, 2 * W], BF16)
            ones128 = small.tile([128, 128], F32)
            ident = small.tile([128, 128], F32)
            dflt = small.tile([BP, 4], F32)

            def emit_early_consts():
                nc.gpsimd.memset(neg_half[:], -0.5)
                nc.gpsimd.memset(neg_thr[:], -ANY_THR)
                nc.gpsimd.memset(b_emp[:], 1.024)
                # OH[p, i*OHW + i] = 1: routes image i to PSUM row i
                nc.gpsimd.memset(ones_oh[:], 1.0)
                nc.gpsimd.affine_select(
                    oh[:], ones_oh[:], [[-1, BP], [1, OHW]], OP.is_equal, 0.0,
                    base=0, channel_multiplier=0,
                )
                # row-index tables, pre-normalized: y = 4p + r;
                # lo = (y-512)/512, hi = -(y+1)/512 (exact in f32)
                nc.gpsimd.iota(hm_lo_i[:], [[0, GROUP], [1, 4]], base=-512,
                               channel_multiplier=4)
                nc.gpsimd.tensor_copy(hm_lo_f[:], hm_lo_i[:])
                nc.gpsimd.tensor_scalar(hm_lo[:], hm_lo_f[:], INV, None, OP.mult)
                nc.gpsimd.iota(hm_hi_i[:], [[0, GROUP], [1, 4]], base=1,
                               channel_multiplier=4)
                nc.gpsimd.tensor_copy(hm_hi_f[:], hm_hi_i[:])
                nc.gpsimd.tensor_scalar(hm_hi[:], hm_hi_f[:], -INV, None, OP.mult)

            def emit_tail_consts():
                # col-index tables (see wm2 comment above)
                nc.gpsimd.iota(wm_i[:, 0:W], [[1, W]], base=-512,
                               channel_multiplier=0)
                nc.gpsimd.iota(wm_i[:, W:2 * W], [[1, W]], base=1,
                               channel_multiplier=0)
                nc.gpsimd.tensor_copy(wm_f[:], wm_i[:])
                nc.gpsimd.tensor_scalar(wm2[:, 0:W], wm_f[:, 0:W], INV, None,
                                        OP.mult)
                nc.gpsimd.tensor_scalar(wm2[:, W:2 * W], wm_f[:, W:2 * W], -INV,
                                        None, OP.mult)
                nc.gpsimd.memset(ones128[:], 1.0)
                nc.gpsimd.affine_select(
                    ident[:], ones128[:], [[-1, 128]], OP.is_equal, 0.0,
                    base=0, channel_multiplier=1,
                )
                nc.gpsimd.memset(dflt[:, 0:2], 0.25)
                nc.gpsimd.memset(dflt[:, 2:4], 0.75)

            # row-side groups; images 30, 31 finish individually
            row_groups = [(0, 8), (8, 8), (16, 8), (24, 6), (30, 1), (31, 1)]

            rT2 = ptr_pool.tile([2 * BP, 128], F32, tag="rT2")
            braw = small.tile([BP, 4], F32)

            def finish_group(start, n):
                cs = slice(4 * start, 4 * (start + n))
                rt_lo = small.tile([128, 4 * GROUP], F32, tag="rt_lo")
                nc.vector.scalar_tensor_tensor(
                    rt_lo[:, 0:4 * n], any_t[:, cs], ANY_THR, hm_lo[:, 0:4 * n],
                    OP.is_gt, OP.mult)
                nc.vector.tensor_reduce(
                    rvals[:, start:start + n],
                    rt_lo[:, 0:4 * n].rearrange("p (i r) -> p i r", r=4),
                    op=OP.min, axis=AX.X)
                rt_hi = small.tile([128, 4 * GROUP], F32, tag="rt_hi")
                nc.vector.scalar_tensor_tensor(
                    rt_hi[:, 0:4 * n], any_t[:, cs], ANY_THR, hm_hi[:, 0:4 * n],
                    OP.is_gt, OP.mult)
                nc.vector.tensor_reduce(
                    rvals[:, BP + start:BP + start + n],
                    rt_hi[:, 0:4 * n].rearrange("p (i r) -> p i r", r=4),
                    op=OP.min, axis=AX.X)

            # per-image row-block engine split: 'd' = DVE is_gt {0,1} count,
            # 'a' = ACT Relu(x-0.5) sum. Image 31: r3 is split in W-halves
            # across DVE+ACT (handled specially in emit_compute).
            def row_engines(i):
                if i == BP - 1:
                    return "ada"  # r0 ACT, r1 DVE, r2 ACT; r3 split
                if i == BP - 2:
                    return "aadd"
                return "ddaa"

            # first image starts with a small piece so the stream begins
            # sooner; last two stream per row-block so tail threshold/matmul
            # latency tracks the last arriving bytes
            def dma_pieces(i):
                if i == 0:
                    return [(0, 1), (1, 4)]
                if i >= BP - 2:
                    return [(0, 1), (1, 2), (2, 3), (3, 4)]
                return None  # one unsliced full-image DMA

            # ACT HWDGE ring for the first triggers: the Sync ring's preamble
            # delays its first trigger ~1us longer than ACT's
            def dma_engine(i):
                return nc.scalar if i <= 4 else nc.sync

            def emit_dma_pair(i):
                # one unsliced 2 MiB DMA covering images i, i+1: per-partition
                # runs stay 8 KiB-coalesced, halves DMA completions
                img2 = imgs.tile([128, 2 * IMG_FREE], F32, tag="imgpair", bufs=6)
                dma_engine(i).dma_start(
                    out=img2[:].rearrange("p (b a w) -> p b a w", b=2, a=4),
                    in_=xv[:, i:i + 2],
                )
                return img2[:, 0:IMG_FREE], img2[:, IMG_FREE:2 * IMG_FREE]

            def emit_dma(i):
                img = imgs.tile([128, IMG_FREE], F32, tag="img", bufs=6)
                pieces = dma_pieces(i)
                if pieces is None:
                    # CRITICAL: unsliced APs. Even a full-range a-dim slice
                    # defeats descriptor coalescing - per-partition runs split
                    # 8 KiB -> 4 x 2 KiB and every DMA engine runs ~12% slower
                    dma_engine(i).dma_start(
                        out=img[:].rearrange("p (a w) -> p a w", a=4),
                        in_=xv[:, i:i + 1],
                    )
                else:
                    img3 = img[:].rearrange("p (a w) -> p a w", a=4)
                    for (al, ah) in pieces:
                        dma_engine(i).dma_start(
                            out=img3[:, al:ah, :],
                            in_=xv[:, i:i + 1, al:ah],
                        )
                return img

            def emit_compute(i, img):
                m01 = masks.tile([128, IMG_FREE], BF16, tag="m01")
                last = i == BP - 1
                engs = row_engines(i)
                for r, eng in enumerate(engs):
                    sl = slice(r * W, (r + 1) * W)
                    acc = any_t[:, 4 * i + r:4 * i + r + 1]
                    if eng == "d":
                        nc.vector.tensor_scalar(
                            m01[:, sl], img[:, sl], 0.5, None,
                            OP.is_gt, OP.add, accum_out=acc)
                    else:
                        nc.scalar.activation(
                            m01[:, sl], img[:, sl], AF.Relu,
                            bias=neg_half[:], accum_out=acc)
                if last:
                    # r3 split in W-halves: DVE low half, ACT high half,
                    # separate accum cells summed on GpSimd
                    sl_a = slice(3 * W, 3 * W + W // 2)
                    sl_b = slice(3 * W + W // 2, 4 * W)
                    nc.vector.tensor_scalar(
                        m01[:, sl_a], img[:, sl_a], 0.5, None,
                        OP.is_gt, OP.add, accum_out=any_ab[:, 0:1])
                    nc.scalar.activation(
                        m01[:, sl_b], img[:, sl_b], AF.Relu,
                        bias=neg_half[:], accum_out=any_ab[:, 1:2])
                    nc.vector.tensor_add(
                        any_t[:, 4 * i + 3:4 * i + 4], any_ab[:, 0:1],
                        any_ab[:, 1:2])
                n_mm = 3 if last else 4
                for r in range(n_mm):
                    sl = slice(r * W, (r + 1) * W)
                    nc.tensor.matmul(
                        psum_col[:, :], oh[:, i * OHW:i * OHW + OHW], m01[:, sl],
                        start=(i == 0 and r == 0), stop=False,
                    )
                if last:
                    # half-width matmuls so each PSUM half gets its stop as
                    # soon as its m01 half lands
                    sl3 = slice(3 * W, 3 * W + W // 2)
                    nc.tensor.matmul(
                        psum_col[:, 0:W // 2], oh[:, i * OHW:i * OHW + OHW],
                        m01[:, sl3], start=False, stop=True,
                    )
                    sl4 = slice(3 * W + W // 2, 4 * W)
                    nc.tensor.matmul(
                        psum_col[:, W // 2:W], oh[:, i * OHW:i * OHW + OHW],
                        m01[:, sl4], start=False, stop=True,
                    )
                for (gs, gn) in row_groups:
                    if gs + gn - 1 == i:
                        finish_group(gs, gn)

            emit_early_consts()
            emit_tail_consts()
            i = 0
            while i < BP:
                if 1 <= i <= 25:
                    a, b = emit_dma_pair(i)
                    emit_compute(i, a)
                    emit_compute(i + 1, b)
                    i += 2
                else:
                    img = emit_dma(i)
                    emit_compute(i, img)
                    i += 1

            # ---- tail ----
            # col side: (psum > thr) in W-halves - DVE is_gt {0,1}, ACT
            # Sign(x-thr) {-1,+1}
            colany = small.tile([BP, W], BF16)
            nc.vector.tensor_scalar(
                colany[:, 0:W // 2], psum_col[0:BP, 0:W // 2], ANY_THR, None,
                OP.is_gt)
            nc.scalar.activation(
                colany[:, W // 2:W], psum_col[0:BP, W // 2:W], AF.Sign,
                bias=neg_thr[0:BP])

            # one (B,2,512) mult: colany broadcast against [wm_lo|wm_hi_neg],
            # then ONE fused min reduce into braw cols {0,2}
            scr = small.tile([BP, 2 * W], BF16)
            cab = colany[:].rearrange("b (one w) -> b one w", one=1)
            nc.vector.tensor_tensor(
                scr[:].rearrange("b (two w) -> b two w", two=2),
                cab.broadcast_to((BP, 2, W)),
                wm2[:].rearrange("b (two w) -> b two w", two=2),
                op=OP.mult)
            nc.vector.tensor_reduce(
                braw[:].rearrange("b (two c) -> b two c", two=2)[:, :, 0:1],
                scr[:].rearrange("b (two w) -> b two w", two=2),
                op=OP.min, axis=AX.X)

            # row side: transpose rvals, reduce per image (PSUM partition
            # slices must be 32-aligned, so full-width reduces)
            nc.tensor.transpose(rT2[:], rvals[:], ident[:])
            nc.vector.tensor_reduce(
                braw[:, 1:2], rT2[0:BP, :], op=OP.min, axis=AX.X)
            nc.vector.tensor_reduce(
                braw[:, 3:4], rT2[BP:2 * BP, :], op=OP.min, axis=AX.X)

            # box algebra on pre-normalized extremes:
            # A = braw[:,0:2] = min-1 side, negB = braw[:,2:4] = -(max+1/512)
            # x0 = A+1, x1 = -negB-1/512, c2 = x0+x1 = A+(1-1/512)-negB
            c2 = small.tile([BP, 2], F32)
            x0 = small.tile([BP, 2], F32)
            x1 = small.tile([BP, 2], F32)
            lo2 = small.tile([BP, 2], F32)
            hi2 = small.tile([BP, 2], F32)
            emp4 = small.tile([BP, 4], I32)  # copy_predicated needs int mask
            final = small.tile([BP, 4], F32)

            # GpSimd per-op dispatch is ~1-2us -> keep it OUT of the tail.
            # Affine helpers go on ACT (parallel to DVE's chain):
            # x0 = A + 1; x1 = -negB - 1/512;
            # emp4 = Relu(1024*negB_y + 1.024) -> int {1 empty, 0 not}
            # (nonempty negB_y <= -1/512 -> arg <= -0.976 -> 0)
            nc.scalar.activation(x0[:], braw[:, 0:2], AF.Copy, bias=1.0)
            nc.scalar.activation(
                x1[:], braw[:, 2:4], AF.Copy, bias=-INV, scale=-1.0)
            nc.scalar.activation(
                emp4[:], braw[:, 3:4].broadcast_to((BP, 4)), AF.Relu,
                bias=b_emp[0:BP], scale=1024.0)
            nc.vector.scalar_tensor_tensor(
                c2[:], braw[:, 0:2], 1.0 - INV, braw[:, 2:4], OP.add, OP.subtract)
            nc.vector.tensor_scalar(
                lo2[:], c2[:], 0.5, MIN_BOX * 0.5, OP.mult, OP.subtract)
            nc.vector.tensor_scalar(
                hi2[:], c2[:], 0.5, MIN_BOX * 0.5, OP.mult, OP.add)
            nc.vector.scalar_tensor_tensor(
                final[:, 0:2], lo2[:], 0.0, x0[:], OP.max, OP.min)
            nc.vector.scalar_tensor_tensor(
                final[:, 2:4], hi2[:], 1.0, x1[:], OP.min, OP.max)

            # default box where empty (one predicated copy)
            nc.vector.copy_predicated(final[:], emp4[:], dflt[:])

            nc.sync.dma_start(out=out.ap(), in_=final[:])

    return nc


_NC = None


def _get_nc():
    global _NC
    if _NC is None:
        nc = build_nc()
        nc.compile()
        _NC = nc
    return _NC


def kernel(mask_fg: np.ndarray) -> np.ndarray:
    mask_fg = np.ascontiguousarray(np.asarray(mask_fg, dtype=np.float32))
    assert mask_fg.shape == (B, 1, H, W), mask_fg.shape
    nc = _get_nc()
    shards = mask_fg.reshape(N_CORES, BP, 1, H, W)
    in_maps = [{"mask_fg": np.ascontiguousarray(shards[i])} for i in range(N_CORES)]
    res = run_bass_kernel_spmd(nc, in_maps, core_ids=list(range(N_CORES)))
    return np.concatenate(
        [res.results[i]["out"] for i in range(N_CORES)], axis=0
    ).astype(np.float32)


# revision 15
# speedup vs baseline: 1.1428x; 1.1428x over previous
"""Trainium2 Bass kernel for nn_BBoxGenerator (segment_reduce).

mask_fg (256, 1, 512, 512) f32 -> boxes (256, 4) f32 [x0, y0, x1, y1].

Pure data parallel: each of the 8 cores handles 32 images independently.

v13 (on top of v12's HWDGE f32 stream):
  Trace analysis of v12 (103-110us): the DMA stream itself sits at the
  per-NC HBM roofline (~94us incl. contention with sibling NCs), but the
  measured window (first framework MEMSET -> last epilogue instruction)
  charges ~2.4us of pre-stream dead time and ~15us after the last input
  byte: compute tail ~5.6us + out-DMA ~1.4us + a fixed ~9us framework
  barrier/sem-reset ceremony. v13 attacks the two movable pieces:

  - Earlier stream start: the first three DMA triggers (image 0 pieces +
    pairs (1,2),(3,4)) issue on the ACT HWDGE ring (nc.scalar) - the Sync
    ring's framework preamble (2nd barrier + 703ns DRAIN) delays its
    first trigger to ~7.2us while ACT can trigger at ~6.2us.  ~1us.
  - Shorter compute tail after the last byte:
    * iota tables pre-scaled by 1/512 (exact in f32; <=1px err in bf16),
      so reduce outputs are already normalized; the +1/-1/512 offsets
      fold into the box algebra. hi-side tables are NEGATED so every
      extreme is a MIN; B = -braw_hi.
    * row side: full transpose T1 + reduces for images 0..29 run
      mid-stream (after group(24,6) at image 29); the tail only
      transposes/reduces images 30,31 (T2, 2 small PSUM reduces).
    * image 31's last row block is thresholded in W-halves on DVE and
      ACT concurrently (two accum cells summed on GpSimd), feeding two
      half-width matmuls; each carries stop for its PSUM columns.
    * col side: (psum > thr) in W-halves - DVE is_gt {0,1} and ACT
      Sign(x-thr) {-1,+1}; with wm_lo<0 and wm_hi_neg<0 the min still
      selects correctly and empty gives 0 (ACT-none contributes positive
      values, ignored by min). One (B,2,512) mult (colany broadcast
      against the combined [wm_lo|wm_hi_neg] table) + ONE fused min
      reduce into braw cols {0,2} (stride-2 out AP).
    * box finishing: c2 = A + (1-1/512) - negB in one stt; lo path on
      DVE, hi path + x0/x1/emp4 on GpSimd; empty-default applied with
      one copy_predicated using a broadcast (B,4) mask.
"""

import numpy as np

from concourse import bacc, mybir
from concourse.tile import TileContext
from concourse.bass_utils import run_bass_kernel_spmd

F32 = mybir.dt.float32
BF16 = mybir.dt.bfloat16
I32 = mybir.dt.int32
OP = mybir.AluOpType
AX = mybir.AxisListType
AF = mybir.ActivationFunctionType

N_CORES = 8
B = 256
BP = B // N_CORES  # 32 images per core
H = W = 512
IMG_FREE = 4 * W  # 2048 free elems per image (4 rows per partition)
GROUP = 8
OHW = 32  # one-hot block width per image (halves PE array power)

MIN_BOX = 0.05
ANY_THR = 0.002  # any_t/PSUM sums exceed this iff any foreground
INV = 1.0 / 512
EMP_THR = 0.001  # nonempty y gives B_y >= 1/512, i.e. negB_y <= -1/512


def build_nc():
    nc = bacc.Bacc("TRN2", target_bir_lowering=False, debug=False, num_devices=N_CORES)
    x = nc.declare_dram_parameter("mask_fg", [BP, 1, H, W], F32, isOutput=False)
    out = nc.declare_dram_parameter("out", [BP, 4], F32, isOutput=True)

    # (128, BP, 4, 512): partition p holds rows 4p..4p+3 of each image
    xv = x.ap().rearrange("b one (p a) w -> p (b one) a w", p=128)

    with TileContext(nc) as tc:
        with (
            tc.tile_pool(name="imgs", bufs=16) as imgs,
            tc.tile_pool(name="masks", bufs=6) as masks,
            tc.tile_pool(name="small", bufs=1) as small,
            tc.tile_pool(name="pcol", bufs=1, space="PSUM") as pcol_pool,
            tc.tile_pool(name="ptr", bufs=2, space="PSUM") as ptr_pool,
        ):
            psum_col = pcol_pool.tile([OHW, W], F32)
            oh = small.tile([128, BP * OHW], BF16)
            ones_oh = small.tile([128, BP * OHW], BF16)
            any_t = small.tile([128, 4 * BP], F32)
            any_ab = small.tile([128, 2], F32)
            rvals = small.tile([128, 2 * BP], F32)

            neg_half = small.tile([128, 1], F32)
            neg_thr = small.tile([128, 1], F32)
            b_emp = small.tile([128, 1], F32)  # emp4 Relu bias (needs AP)
            hm_lo_i = small.tile([128, GROUP * 4], I32)
            hm_lo_f = small.tile([128, GROUP * 4], F32)
            hm_lo = small.tile([128, GROUP * 4], F32)
            hm_hi_i = small.tile([128, GROUP * 4], I32)
            hm_hi_f = small.tile([128, GROUP * 4], F32)
            hm_hi = small.tile([128, GROUP * 4], F32)
            # combined col-index table: [:, 0:W] = (j-512)/512 (lo),
            # [:, W:2W] = -(j+1)/512 (negated hi); both < 0
            wm_i = small.tile([BP, 2 * W], I32)
            wm_f = small.tile([BP, 2 * W], F32)
            wm2 = small.tile([BP, 2 * W], BF16)
            ones128 = small.tile([128, 128], F32)
            ident = small.tile([128, 128], F32)
            dflt = small.tile([BP, 4], F32)

            def emit_early_consts():
                nc.gpsimd.memset(neg_half[:], -0.5)
                nc.gpsimd.memset(neg_thr[:], -ANY_THR)
                nc.gpsimd.memset(b_emp[:], 1.024)
                # OH[p, i*OHW + i] = 1: routes image i to PSUM row i
                nc.gpsimd.memset(ones_oh[:], 1.0)
                nc.gpsimd.affine_select(
                    oh[:], ones_oh[:], [[-1, BP], [1, OHW]], OP.is_equal, 0.0,
                    base=0, channel_multiplier=0,
                )
                # row-index tables, pre-normalized: y = 4p + r;
                # lo = (y-512)/512, hi = -(y+1)/512 (exact in f32)
                nc.gpsimd.iota(hm_lo_i[:], [[0, GROUP], [1, 4]], base=-512,
                               channel_multiplier=4)
                nc.gpsimd.tensor_copy(hm_lo_f[:], hm_lo_i[:])
                nc.gpsimd.tensor_scalar(hm_lo[:], hm_lo_f[:], INV, None, OP.mult)
                nc.gpsimd.iota(hm_hi_i[:], [[0, GROUP], [1, 4]], base=1,
                               channel_multiplier=4)
                nc.gpsimd.tensor_copy(hm_hi_f[:], hm_hi_i[:])
                nc.gpsimd.tensor_scalar(hm_hi[:], hm_hi_f[:], -INV, None, OP.mult)

            def emit_tail_consts():
                # col-index tables (see wm2 comment above)
                nc.gpsimd.iota(wm_i[:, 0:W], [[1, W]], base=-512,
                               channel_multiplier=0)
                nc.gpsimd.iota(wm_i[:, W:2 * W], [[1, W]], base=1,
                               channel_multiplier=0)
                nc.gpsimd.tensor_copy(wm_f[:], wm_i[:])
                nc.gpsimd.tensor_scalar(wm2[:, 0:W], wm_f[:, 0:W], INV, None,
                                        OP.mult)
                nc.gpsimd.tensor_scalar(wm2[:, W:2 * W], wm_f[:, W:2 * W], -INV,
                                        None, OP.mult)
                nc.gpsimd.memset(ones128[:], 1.0)
                nc.gpsimd.affine_select(
                    ident[:], ones128[:], [[-1, 128]], OP.is_equal, 0.0,
                    base=0, channel_multiplier=1,
                )
                nc.gpsimd.memset(dflt[:, 0:2], 0.25)
                nc.gpsimd.memset(dflt[:, 2:4], 0.75)

            # row-side groups; images 30, 31 finish individually
            row_groups = [(0, 8), (8, 8), (16, 8), (24, 6), (30, 1), (31, 1)]

            rT2 = ptr_pool.tile([2 * BP, 128], F32, tag="rT2")
            braw = small.tile([BP, 4], F32)

            def finish_group(start, n):
                cs = slice(4 * start, 4 * (start + n))
                rt_lo = small.tile([128, 4 * GROUP], F32, tag="rt_lo")
                nc.vector.scalar_tensor_tensor(
                    rt_lo[:, 0:4 * n], any_t[:, cs], ANY_THR, hm_lo[:, 0:4 * n],
                    OP.is_gt, OP.mult)
                nc.vector.tensor_reduce(
                    rvals[:, start:start + n],
                    rt_lo[:, 0:4 * n].rearrange("p (i r) -> p i r", r=4),
                    op=OP.min, axis=AX.X)
                rt_hi = small.tile([128, 4 * GROUP], F32, tag="rt_hi")
                nc.vector.scalar_tensor_tensor(
                    rt_hi[:, 0:4 * n], any_t[:, cs], ANY_THR, hm_hi[:, 0:4 * n],
                    OP.is_gt, OP.mult)
                nc.vector.tensor_reduce(
                    rvals[:, BP + start:BP + start + n],
                    rt_hi[:, 0:4 * n].rearrange("p (i r) -> p i r", r=4),
                    op=OP.min, axis=AX.X)

            # per-image row-block engine split: 'd' = DVE is_gt {0,1} count,
            # 'a' = ACT Relu(x-0.5) sum. Image 31: r3 is split in W-halves
            # across DVE+ACT (handled specially in emit_compute).
            def row_engines(i):
                if i == BP - 1:
                    return "ada"  # r0 ACT, r1 DVE, r2 ACT; r3 split
                if i == BP - 2:
                    return "aadd"
                return "ddaa"

            # first image starts with a small piece so the stream begins
            # sooner; last two stream per row-block so tail threshold/matmul
            # latency tracks the last arriving bytes
            def dma_pieces(i):
                if i == 0:
                    return [(0, 1), (1, 4)]
                if i >= BP - 2:
                    return [(0, 1), (1, 2), (2, 3), (3, 4)]
                return None  # one unsliced full-image DMA

            # ACT HWDGE ring for the first triggers: the Sync ring's preamble
            # delays its first trigger ~1us longer than ACT's
            def dma_engine(i):
                return nc.scalar if i <= 4 else nc.sync

            def emit_dma_pair(i):
                # one unsliced 2 MiB DMA covering images i, i+1: per-partition
                # runs stay 8 KiB-coalesced, halves DMA completions
                img2 = imgs.tile([128, 2 * IMG_FREE], F32, tag="imgpair", bufs=6)
                dma_engine(i).dma_start(
                    out=img2[:].rearrange("p (b a w) -> p b a w", b=2, a=4),
                    in_=xv[:, i:i + 2],
                )
                return img2[:, 0:IMG_FREE], img2[:, IMG_FREE:2 * IMG_FREE]

            def emit_dma(i):
                img = imgs.tile([128, IMG_FREE], F32, tag="img", bufs=6)
                pieces = dma_pieces(i)
                if pieces is None:
                    # CRITICAL: unsliced APs. Even a full-range a-dim slice
                    # defeats descriptor coalescing - per-partition runs split
                    # 8 KiB -> 4 x 2 KiB and every DMA engine runs ~12% slower
                    dma_engine(i).dma_start(
                        out=img[:].rearrange("p (a w) -> p a w", a=4),
                        in_=xv[:, i:i + 1],
                    )
                else:
                    img3 = img[:].rearrange("p (a w) -> p a w", a=4)
                    for (al, ah) in pieces:
                        dma_engine(i).dma_start(
                            out=img3[:, al:ah, :],
                            in_=xv[:, i:i + 1, al:ah],
                        )
                return img

            def emit_compute(i, img):
                m01 = masks.tile([128, IMG_FREE], BF16, tag="m01", bufs=8)
                last = i == BP - 1
                engs = row_engines(i)
                for r, eng in enumerate(engs):
                    sl = slice(r * W, (r + 1) * W)
                    acc = any_t[:, 4 * i + r:4 * i + r + 1]
                    if eng == "d":
                        nc.vector.tensor_scalar(
                            m01[:, sl], img[:, sl], 0.5, None,
                            OP.is_gt, OP.add, accum_out=acc)
                    else:
                        nc.scalar.activation(
                            m01[:, sl], img[:, sl], AF.Relu,
                            bias=neg_half[:], accum_out=acc)
                if last:
                    # r3 split in W-halves: DVE low half, ACT high half,
                    # separate accum cells summed on GpSimd
                    sl_a = slice(3 * W, 3 * W + W // 2)
                    sl_b = slice(3 * W + W // 2, 4 * W)
                    nc.vector.tensor_scalar(
                        m01[:, sl_a], img[:, sl_a], 0.5, None,
                        OP.is_gt, OP.add, accum_out=any_ab[:, 0:1])
                    nc.scalar.activation(
                        m01[:, sl_b], img[:, sl_b], AF.Relu,
                        bias=neg_half[:], accum_out=any_ab[:, 1:2])
                    nc.vector.tensor_add(
                        any_t[:, 4 * i + 3:4 * i + 4], any_ab[:, 0:1],
                        any_ab[:, 1:2])
                n_mm = 3 if last else 4
                for r in range(n_mm):
                    sl = slice(r * W, (r + 1) * W)
                    nc.tensor.matmul(
                        psum_col[:, :], oh[:, i * OHW:i * OHW + OHW], m01[:, sl],
                        start=(i == 0 and r == 0), stop=False,
                    )
                if last:
                    # half-width matmuls so each PSUM half gets its stop as
                    # soon as its m01 half lands
                    sl3 = slice(3 * W, 3 * W + W // 2)
                    nc.tensor.matmul(
                        psum_col[:, 0:W // 2], oh[:, i * OHW:i * OHW + OHW],
                        m01[:, sl3], start=False, stop=True,
                    )
                    sl4 = slice(3 * W + W // 2, 4 * W)
                    nc.tensor.matmul(
                        psum_col[:, W // 2:W], oh[:, i * OHW:i * OHW + OHW],
                        m01[:, sl4], start=False, stop=True,
                    )
                for (gs, gn) in row_groups:
                    if gs + gn - 1 == i:
                        finish_group(gs, gn)

            emit_early_consts()
            emit_tail_consts()
            i = 0
            while i < BP:
                if 1 <= i <= 25:
                    a, b = emit_dma_pair(i)
                    emit_compute(i, a)
                    emit_compute(i + 1, b)
                    i += 2
                else:
                    img = emit_dma(i)
                    emit_compute(i, img)
                    i += 1

            # ---- tail ----
            # col side: (psum > thr) in W-halves - DVE is_gt {0,1}, ACT
            # Sign(x-thr) {-1,+1}
            colany = small.tile([BP, W], BF16)
            nc.vector.tensor_scalar(
                colany[:, 0:W // 2], psum_col[0:BP, 0:W // 2], ANY_THR, None,
                OP.is_gt)
            nc.scalar.activation(
                colany[:, W // 2:W], psum_col[0:BP, W // 2:W], AF.Sign,
                bias=neg_thr[0:BP])

            # one (B,2,512) mult: colany broadcast against [wm_lo|wm_hi_neg],
            # then ONE fused min reduce into braw cols {0,2}
            scr = small.tile([BP, 2 * W], BF16)
            cab = colany[:].rearrange("b (one w) -> b one w", one=1)
            nc.vector.tensor_tensor(
                scr[:].rearrange("b (two w) -> b two w", two=2),
                cab.broadcast_to((BP, 2, W)),
                wm2[:].rearrange("b (two w) -> b two w", two=2),
                op=OP.mult)
            nc.vector.tensor_reduce(
                braw[:].rearrange("b (two c) -> b two c", two=2)[:, :, 0:1],
                scr[:].rearrange("b (two w) -> b two w", two=2),
                op=OP.min, axis=AX.X)

            # row side: transpose rvals, reduce per image (PSUM partition
            # slices must be 32-aligned, so full-width reduces)
            nc.tensor.transpose(rT2[:], rvals[:], ident[:])
            nc.vector.tensor_reduce(
                braw[:, 1:2], rT2[0:BP, :], op=OP.min, axis=AX.X)
            nc.vector.tensor_reduce(
                braw[:, 3:4], rT2[BP:2 * BP, :], op=OP.min, axis=AX.X)

            # box algebra on pre-normalized extremes:
            # A = braw[:,0:2] = min-1 side, negB = braw[:,2:4] = -(max+1/512)
            # x0 = A+1, x1 = -negB-1/512, c2 = x0+x1 = A+(1-1/512)-negB
            c2 = small.tile([BP, 2], F32)
            x0 = small.tile([BP, 2], F32)
            x1 = small.tile([BP, 2], F32)
            lo2 = small.tile([BP, 2], F32)
            hi2 = small.tile([BP, 2], F32)
            emp4 = small.tile([BP, 4], I32)  # copy_predicated needs int mask
            final = small.tile([BP, 4], F32)

            # GpSimd per-op dispatch is ~1-2us -> keep it OUT of the tail.
            # Affine helpers go on ACT (parallel to DVE's chain):
            # x0 = A + 1; x1 = -negB - 1/512;
            # emp4 = Relu(1024*negB_y + 1.024) -> int {1 empty, 0 not}
            # (nonempty negB_y <= -1/512 -> arg <= -0.976 -> 0)
            nc.scalar.activation(x0[:], braw[:, 0:2], AF.Copy, bias=1.0)
            nc.scalar.activation(
                x1[:], braw[:, 2:4], AF.Copy, bias=-INV, scale=-1.0)
            nc.scalar.activation(
                emp4[:], braw[:, 3:4].broadcast_to((BP, 4)), AF.Relu,
                bias=b_emp[0:BP], scale=1024.0)
            nc.vector.scalar_tensor_tensor(
                c2[:], braw[:, 0:2], 1.0 - INV, braw[:, 2:4], OP.add, OP.subtract)
            nc.vector.tensor_scalar(
                lo2[:], c2[:], 0.5, MIN_BOX * 0.5, OP.mult, OP.subtract)
            nc.vector.tensor_scalar(
                hi2[:], c2[:], 0.5, MIN_BOX * 0.5, OP.mult, OP.add)
            nc.vector.scalar_tensor_tensor(
                final[:, 0:2], lo2[:], 0.0, x0[:], OP.max, OP.min)
            nc.vector.scalar_tensor_tensor(
                final[:, 2:4], hi2[:], 1.0, x1[:], OP.min, OP.max)

            # default box where empty (one predicated copy)
            nc.vector.copy_predicated(final[:], emp4[:], dflt[:])

            nc.sync.dma_start(out=out.ap(), in_=final[:])

    return nc


_NC = None


def _get_nc():
    global _NC
    if _NC is None:
        nc = build_nc()
        nc.compile()
        _NC = nc
    return _NC


def kernel(mask_fg: np.ndarray) -> np.ndarray:
    mask_fg = np.ascontiguousarray(np.asarray(mask_fg, dtype=np.float32))
    assert mask_fg.shape == (B, 1, H, W), mask_fg.shape
    nc = _get_nc()
    shards = mask_fg.reshape(N_CORES, BP, 1, H, W)
    in_maps = [{"mask_fg": np.ascontiguousarray(shards[i])} for i in range(N_CORES)]
    res = run_bass_kernel_spmd(nc, in_maps, core_ids=list(range(N_CORES)))
    return np.concatenate(
        [res.results[i]["out"] for i in range(N_CORES)], axis=0
    ).astype(np.float32)
